# revision 22
# baseline (speedup 1.0000x reference)
"""ARD RBF Gram matrix kernel for Trainium2 (8 NeuronCores, SPMD).

K[i, j] = exp(-0.5 * sum_d (x[i,d] - y[j,d])^2 / exp(logh[d]))

Sharding: 2x4 core grid. Core c = (r, q) with r = c // 4, q = c % 4 owns the
output block rows [r*4096, (r+1)*4096) x cols [q*2048, (q+1)*2048). This
minimizes per-core input DMA (8MB of x + 4MB of y vs 2MB + 16MB for pure row
sharding).

Device-side algorithm per core, using the factorization
  K = exp(-0.5*cross - 0.5*x2[i]) * exp(-0.5*y2[j]),  cross = -2 sum ih^2 x y:

  ih      = exp(-0.5 * logh)                       (ACT)
  xs8     = fp8e4(x^T * ih)     [d, c, i] layout   (ACT/DVE per-part. scale)
  ys8     = fp8e4(y^T * -2ih)   [d, c, j] layout
  mhx2[i] = -0.5*sum_d ih^2 x^2  (DVE bf16 square + bf16 reduce matmuls;
                                  becomes the ACT bias via transpose DMAs)
  ey2[j]  = exp(-0.5*sum_d ih^2 y^2)  (ACT exp), replicated to all 128
                                  partitions via ones-matmuls
  psum    = cross  (fp8 DoubleRow matmuls, 256-deep contraction per pass)
  tmp     = exp(-0.5*psum + mhx2[i])   (ACT, PSUM -> SBUF fp16)
  out     = tmp * ey2rep               (DVE fp16 multiply)
  DMA store fp16 to DRAM; host widens fp16 -> fp32 (lossless).

Engine-cost rules learned from traces on this hardware:
  - DVE/Pool elementwise ops have ~1us fixed cost; DVE streams f32/f16 at
    ~2-3 elem/lane/cycle but fp8 writes at ~0.6; Pool is ~2x slower than DVE
    at everything. So: fewest/widest ops, fp8 conversions split ACT (prep) /
    DVE (main-phase slack), squares in bf16 on DVE, tiny row copies on Pool.
  - PE fp8 DoubleRow matmuls stream 2 fp8 columns/cycle; the PE stream must
    stay gap-free or the HAM p-state throttles to ~1.2-1.3 GHz.
  - x-slab 1's fp8 conversion is software-pipelined into the first main-loop
    iterations (its itiles run last), so the main loop starts after only
    y + x-slab-0 prep.

The host side only reshapes/transposes/shards numpy arrays and losslessly
widens the fp16 result; every value-changing floating point operation
happens on device.
"""

import json

import numpy as np

import concourse.bass as bass
import concourse.mybir as mybir
import concourse.tile as tile
from concourse.bass_utils import run_bass_kernel_spmd

N_CORES = 8
N, M, D = 8192, 8192, 512
RG, CG = 2, 4  # core grid: RG row groups x CG col groups
NI = N // RG  # x rows per core (4096)
MJ = M // CG  # y cols per core (2048)
P = 128  # partitions
NCHUNK = D // P  # contraction chunks (4)
NPAIR = NCHUNK // 2  # fp8 DoubleRow chunk pairs (2)
ITILES = NI // P  # i tiles per core (32)
SLABW = 2048  # prep slab width

F32 = mybir.dt.float32
F32R = mybir.dt.float32r
BF16 = mybir.dt.bfloat16
F16 = mybir.dt.float16
FP8 = mybir.dt.float8e4
AF = mybir.ActivationFunctionType
DR = mybir.MatmulPerfMode.DoubleRow

# ---------------------------------------------------------------------------
# Workaround for this walrus build: only ONE sync-wait condition is allowed
# per instruction ("Too many sync wait commands"). Split excess on_wait
# entries onto preceding NoOps on the same engine (program order preserves
# semantics exactly).
# ---------------------------------------------------------------------------
_WAIT_LIMIT = 1


def _split_excess_waits(bir: dict, limit: int = _WAIT_LIMIT) -> dict:
    # Excess waits are moved onto preceding EventSemaphore instructions,
    # which this walrus accepts with up to TWO wait conditions (ordinary
    # instructions allow only one). Program order preserves semantics.
    counter = 0
    for fn in bir.get("functions", []):
        for bb in fn.get("blocks", []):
            new_insts = []
            for inst in bb.get("instructions", []):
                si = inst.get("sync_info")
                waits = si.get("on_wait") if si else None
                eng = inst.get("engine", "Unassigned")
                if waits and len(waits) > limit and eng != "Unassigned":
                    keep = len(waits) % 2  # odd count: last wait stays put
                    head = waits[: len(waits) - keep]
                    for i in range(0, len(head), 2):
                        counter += 1
                        new_insts.append(
                            {
                                "debug": inst.get("debug", 0),
                                "engine": eng,
                                "ins": [],
                                "outs": [],
                                "name": f"WS-{counter}-{inst['name']}",
                                "opcode": "EventSemaphore",
                                "sync_info": {
                                    "on_update": [],
                                    "on_wait": head[i : i + 2],
                                },
                            }
                        )
                    si["on_wait"] = waits[len(waits) - keep :]
                new_insts.append(inst)
            bb["instructions"] = new_insts
    return bir


def _patch_nc(nc):
    orig = nc.to_json_bytes

    def patched() -> bytes:
        return json.dumps(_split_excess_waits(json.loads(orig()))).encode()

    nc.to_json_bytes = patched
    return nc


# ---------------------------------------------------------------------------
# Device program (identical on all 8 cores; only DRAM contents differ)
# ---------------------------------------------------------------------------


def _build_nc():
    nc = bass.Bass()

    xt = nc.dram_tensor("xt", [D, NI], F32, kind="ExternalInput")
    yt = nc.dram_tensor("yt", [D, MJ], F32, kind="ExternalInput")
    lh = nc.dram_tensor("lh", [NCHUNK, P], F32, kind="ExternalInput")
    out = nc.dram_tensor("out", [NI, MJ], F16, kind="ExternalOutput")

    xt_r = xt.rearrange("(c d) i -> d c i", d=P)
    yt_r = yt.rearrange("(c d) j -> d c j", d=P)

    with tile.TileContext(nc) as tc:
        with (
            tc.tile_pool(name="singles", bufs=1) as singles,
            tc.tile_pool(name="stage", bufs=6) as stage,
            tc.tile_pool(name="sqp", bufs=2) as sqp,
            tc.tile_pool(name="outp", bufs=3) as outp,
            tc.tile_pool(name="tmpp", bufs=3) as tmpp,
        ):
            # persistent SBUF tensors
            # pair-grouped DR layouts: [P, block, pair-group, 2, width]
            # so the DoubleRow pair stride is 128/512 bytes, not 2048
            xs8 = singles.tile([P, ITILES, NPAIR, 2, P], FP8)
            ys8 = singles.tile([P, MJ // 512, NPAIR, 2, 512], FP8)
            mhx2a = singles.tile([P, ITILES // 2], F32)  # -0.5*x2 bias, slab0
            mhx2b = singles.tile([P, ITILES // 2], F32)  # ... slab 1
            sxa = singles.tile([1, SLABW], F32)  # -0.5 * x2 row, slab 0
            sxb = singles.tile([1, SLABW], F32)  # ... slab 1
            ey2row = singles.tile([1, MJ], BF16)  # exp(-0.5*y2) row
            ey2rep = singles.tile([P, MJ], F16)  # ... replicated
            ones1 = singles.tile([1, P], BF16)  # replicate-matmul lhsT
            lhs = singles.tile([P, NCHUNK], F32)
            ih = singles.tile([P, NCHUNK], F32)
            ihm2 = singles.tile([P, NCHUNK], F32)
            ihsq = singles.tile([P, NCHUNK], BF16)  # ih^2 reduce lhsT
            mihsq = singles.tile([P, NCHUNK], BF16)  # -0.5 ih^2 reduce lhsT

            nc.sync.dma_start(out=lhs, in_=lh.rearrange("c d -> d c"))
            nc.scalar.activation(ih, lhs, AF.Exp, scale=-0.5)
            nc.vector.tensor_scalar_mul(ihm2, ih, -2.0)
            nc.vector.tensor_mul(ihsq, ih, ih)
            nc.vector.tensor_scalar_mul(mihsq, ihsq, -0.5)
            nc.vector.memset(ones1, 1.0)

            held = {}  # x-slab-1 f32 chunk tiles, converted in main phase

            # ---- prep: loads, bf16 squares (DVE), bf16 row-reduce matmuls,
            # fp8 conversions (ACT; x-slab 1 deferred to the main phase) ----
            with tc.tile_pool(name="accp", bufs=1, space="PSUM") as accp:

                def prep_slab(src_r, s0, dst8, scale, lhsT, pfx, defer):
                    accs = [
                        accp.tile([1, 512], F32, tag=f"a{js}", name=f"{pfx}a{js}")
                        for js in range(SLABW // 512)
                    ]
                    for c in range(NCHUNK):
                        sf = stage.tile(
                            [P, SLABW], F32, tag="sf", name=f"{pfx}f{c}"
                        )
                        nc.sync.dma_start(
                            out=sf, in_=src_r[:, c, s0 : s0 + SLABW]
                        )
                        sq = sqp.tile(
                            [P, SLABW], BF16, tag="sq", name=f"{pfx}sq{c}"
                        )
                        nc.vector.tensor_mul(sq, sf, sf)
                        for js in range(SLABW // 512):
                            nc.tensor.matmul(
                                accs[js],
                                lhsT[:, c : c + 1],
                                sq[:, js * 512 : (js + 1) * 512],
                                start=(c == 0),
                                stop=(c == NCHUNK - 1),
                            )
                        if defer:
                            held[c] = sf
                        else:
                            b0 = s0 // dst8.shape[-1]
                            nb = SLABW // dst8.shape[-1]
                            nc.scalar.mul(
                                dst8[:, b0 : b0 + nb, c // 2, c % 2, :],
                                sf,
                                scale[:, c : c + 1],
                            )
                    return accs

                # y: ys8 + exp(-0.5*y2) row
                accs = prep_slab(yt_r, 0, ys8, ihm2, ihsq, "y", False)
                for js, acc in enumerate(accs):
                    nc.scalar.activation(
                        ey2row[0:1, js * 512 : (js + 1) * 512],
                        acc,
                        AF.Exp,
                        scale=-0.5,
                    )

                # x slab 0: xs8 + -0.5*x2 row
                accs = prep_slab(xt_r, 0, xs8, ih, mihsq, "x0", False)
                for js, acc in enumerate(accs):
                    nc.scalar.copy(sxa[0:1, js * 512 : (js + 1) * 512], acc)

                # x slab 1: loads only — squares/reduce/conversion all run
                # in the main phase so accp can close (freeing its PSUM
                # banks) as soon as y + x-slab-0 are reduced, letting the
                # main loop start ~14us earlier.
                for c in range(NCHUNK):
                    sf = stage.tile([P, SLABW], F32, tag="sf", name=f"x1f{c}")
                    nc.sync.dma_start(
                        out=sf, in_=xt_r[:, c, SLABW : 2 * SLABW]
                    )
                    held[c] = sf

            # transpose -0.5*x2 row (slab 0) -> [P, 16] for the ACT bias:
            # one column DMA per itile (anything wider needs >3 AP dims,
            # which the DMA engine can't express).
            for it in range(ITILES // 2):
                nc.sync.dma_start(
                    out=mhx2a[:, it : it + 1],
                    in_=sxa[0:1, it * P : (it + 1) * P],
                )

            sq1 = [
                singles.tile([P, SLABW], BF16, name=f"sq1_{c}")
                for c in range(NCHUNK)
            ]

            with tc.tile_pool(name="mainps", bufs=2, space="PSUM") as mainps:
                # replicate ey2row to all 128 partitions with ones-matmuls
                rep = mainps.tile([P, MJ], F32, tag="ps", name="rep")
                for h in range(MJ // 512):
                    nc.tensor.matmul(
                        rep[:, h * 512 : (h + 1) * 512],
                        ones1,
                        ey2row[0:1, h * 512 : (h + 1) * 512],
                        start=True,
                        stop=True,
                    )
                nc.vector.tensor_copy(ey2rep, rep)

                # ---- main loop: fp8 DR matmuls, ACT exp, DVE scale; the
                # x-slab-1 prep (fp8 conversion, squares, x2 reduction into
                # a borrowed PSUM ring slot) is software-pipelined into the
                # first itiles, whose outputs only need slab 0 ----
                for it in range(ITILES):
                    if 2 <= it < 2 + NCHUNK:
                        c = it - 2
                        nc.vector.tensor_scalar_mul(
                            xs8[:, ITILES // 2 :, c // 2, c % 2, :],
                            held[c],
                            ih[:, c : c + 1],
                        )
                        nc.gpsimd.tensor_mul(sq1[c], held[c], held[c])
                    if it == 2 + NCHUNK:
                        # -0.5*x2 for slab 1, reduced in a borrowed slot
                        xacc = mainps.tile([P, MJ], F32, tag="ps", name="xacc")
                        for js in range(SLABW // 512):
                            jsl = slice(js * 512, (js + 1) * 512)
                            for c in range(NCHUNK):
                                nc.tensor.matmul(
                                    xacc[0:1, jsl],
                                    mihsq[:, c : c + 1],
                                    sq1[c][:, jsl],
                                    start=(c == 0),
                                    stop=(c == NCHUNK - 1),
                                )
                            nc.scalar.copy(sxb[0:1, jsl], xacc[0:1, jsl])
                        for itc2 in range(ITILES // 2):
                            nc.sync.dma_start(
                                out=mhx2b[:, itc2 : itc2 + 1],
                                in_=sxb[0:1, itc2 * P : (itc2 + 1) * P],
                            )
                    isl = slice(it * P, (it + 1) * P)
                    mhx2 = mhx2a if it < ITILES // 2 else mhx2b
                    itc = it % (ITILES // 2)
                    ps = mainps.tile([P, MJ], F32, tag="ps", name=f"ps{it}")
                    for t in range(NPAIR):
                        csl = slice(2 * t, 2 * t + 2)
                        for js in range(MJ // 512):
                            j0 = js * 512
                            nc.tensor.matmul(
                                ps[:, j0 : j0 + 512],
                                xs8[:, it, t, :, :],
                                ys8[:, js, t, :, :],
                                start=(t == 0),
                                stop=(t == NPAIR - 1),
                                perf_mode=DR,
                            )
                    tmp = tmpp.tile([P, MJ], F16, tag="tmp", name=f"t{it}")
                    nc.scalar.activation(
                        tmp,
                        ps,
                        AF.Exp,
                        bias=mhx2[:, itc : itc + 1],
                        scale=-0.5,
                    )
                    ot = outp.tile([P, MJ], F16, tag="ot", name=f"ot{it}")
                    nc.vector.tensor_mul(ot, tmp, ey2rep)
                    nc.sync.dma_start(out=out[isl, :], in_=ot)

    return _patch_nc(nc)


_NC_CACHE = None

# test.py hooks: set _TRACE to capture a profile; results object stored here.
_TRACE = False
_TRACE_KWARGS = {}
LAST_RESULTS = None


def kernel(x, y, logh):
    global _NC_CACHE, LAST_RESULTS
    x = np.ascontiguousarray(np.asarray(x, dtype=np.float32))
    y = np.ascontiguousarray(np.asarray(y, dtype=np.float32))
    logh = np.ascontiguousarray(np.asarray(logh, dtype=np.float32))
    assert x.shape == (N, D) and y.shape == (M, D) and logh.shape == (D,)

    if _NC_CACHE is None:
        _NC_CACHE = _build_nc()
    nc = _NC_CACHE

    lhm = np.ascontiguousarray(logh.reshape(NCHUNK, P))
    xts = [
        np.ascontiguousarray(x[r * NI : (r + 1) * NI, :].T) for r in range(RG)
    ]
    yts = [
        np.ascontiguousarray(y[q * MJ : (q + 1) * MJ, :].T) for q in range(CG)
    ]
    in_maps = []
    for c in range(N_CORES):
        r, q = divmod(c, CG)
        in_maps.append({"xt": xts[r], "yt": yts[q], "lh": lhm})

    res = run_bass_kernel_spmd(
        nc,
        in_maps,
        core_ids=list(range(N_CORES)),
        trace=_TRACE,
        **_TRACE_KWARGS,
    )
    LAST_RESULTS = res
    full = np.empty((N, M), dtype=np.float32)
    for c in range(N_CORES):
        r, q = divmod(c, CG)
        full[r * NI : (r + 1) * NI, q * MJ : (q + 1) * MJ] = res.results[c][
            "out"
        ].astype(np.float32)
    return full


# revision 23
# speedup vs baseline: 1.1547x; 1.1547x over previous
"""ARD RBF Gram matrix kernel for Trainium2 (8 NeuronCores, SPMD).

K[i, j] = exp(-0.5 * sum_d (x[i,d] - y[j,d])^2 / exp(logh[d]))

Sharding: 2x4 core grid. Core c = (r, q) with r = c // 4, q = c % 4 owns the
output block rows [r*4096, (r+1)*4096) x cols [q*2048, (q+1)*2048). This
minimizes per-core input DMA (8MB of x + 4MB of y vs 2MB + 16MB for pure row
sharding).

Device-side algorithm per core, using the factorization
  K = exp(-0.5*cross - 0.5*x2[i]) * exp(-0.5*y2[j]),  cross = -2 sum ih^2 x y:

  ih      = exp(-0.5 * logh)                       (ACT)
  xs8     = fp8e4(x^T * ih)     [d, c, i] layout   (ACT/DVE per-part. scale)
  ys8     = fp8e4(y^T * -2ih)   [d, c, j] layout
  mhx2[i] = -0.5*sum_d ih^2 x^2  (DVE bf16 square + bf16 reduce matmuls;
                                  becomes the ACT bias via transpose DMAs)
  ey2[j]  = exp(-0.5*sum_d ih^2 y^2)  (ACT exp), replicated to all 128
                                  partitions via ones-matmuls
  psum    = cross  (fp8 DoubleRow matmuls, 256-deep contraction per pass)
  tmp     = exp(-0.5*psum + mhx2[i])   (ACT, PSUM -> SBUF fp16)
  out     = tmp * ey2rep               (DVE fp16 multiply)
  DMA store fp16 to DRAM; host widens fp16 -> fp32 (lossless).

Engine-cost rules learned from traces on this hardware:
  - DVE/Pool elementwise ops have ~1us fixed cost; DVE streams f32/f16 at
    ~2-3 elem/lane/cycle but fp8 writes at ~0.6; Pool is ~2x slower than DVE
    at everything. So: fewest/widest ops, fp8 conversions split ACT (prep) /
    DVE (main-phase slack), squares in bf16 on DVE, tiny row copies on Pool.
  - PE fp8 DoubleRow matmuls stream 2 fp8 columns/cycle; the PE stream must
    stay gap-free or the HAM p-state throttles to ~1.2-1.3 GHz.
  - x-slab 1's fp8 conversion is software-pipelined into the first main-loop
    iterations (its itiles run last), so the main loop starts after only
    y + x-slab-0 prep.

The host side only reshapes/transposes/shards numpy arrays and losslessly
widens the fp16 result; every value-changing floating point operation
happens on device.
"""

import json

import numpy as np

import concourse.bass as bass
import concourse.mybir as mybir
import concourse.tile as tile
from concourse.bass_utils import run_bass_kernel_spmd

N_CORES = 8
N, M, D = 8192, 8192, 512
RG, CG = 2, 4  # core grid: RG row groups x CG col groups
NI = N // RG  # x rows per core (4096)
MJ = M // CG  # y cols per core (2048)
P = 128  # partitions
NCHUNK = D // P  # contraction chunks (4)
NPAIR = NCHUNK // 2  # fp8 DoubleRow chunk pairs (2)
ITILES = NI // P  # i tiles per core (32)
SLABW = 2048  # prep slab width

F32 = mybir.dt.float32
F32R = mybir.dt.float32r
BF16 = mybir.dt.bfloat16
F16 = mybir.dt.float16
FP8 = mybir.dt.float8e4
AF = mybir.ActivationFunctionType
DR = mybir.MatmulPerfMode.DoubleRow

# ---------------------------------------------------------------------------
# Workaround for this walrus build: only ONE sync-wait condition is allowed
# per instruction ("Too many sync wait commands"). Split excess on_wait
# entries onto preceding NoOps on the same engine (program order preserves
# semantics exactly).
# ---------------------------------------------------------------------------
_WAIT_LIMIT = 1


def _split_excess_waits(bir: dict, limit: int = _WAIT_LIMIT) -> dict:
    # Excess waits are moved onto preceding EventSemaphore instructions,
    # which this walrus accepts with up to TWO wait conditions (ordinary
    # instructions allow only one). Program order preserves semantics.
    counter = 0
    for fn in bir.get("functions", []):
        for bb in fn.get("blocks", []):
            new_insts = []
            for inst in bb.get("instructions", []):
                si = inst.get("sync_info")
                waits = si.get("on_wait") if si else None
                eng = inst.get("engine", "Unassigned")
                if waits and len(waits) > limit and eng != "Unassigned":
                    keep = len(waits) % 2  # odd count: last wait stays put
                    head = waits[: len(waits) - keep]
                    for i in range(0, len(head), 2):
                        counter += 1
                        new_insts.append(
                            {
                                "debug": inst.get("debug", 0),
                                "engine": eng,
                                "ins": [],
                                "outs": [],
                                "name": f"WS-{counter}-{inst['name']}",
                                "opcode": "EventSemaphore",
                                "sync_info": {
                                    "on_update": [],
                                    "on_wait": head[i : i + 2],
                                },
                            }
                        )
                    si["on_wait"] = waits[len(waits) - keep :]
                new_insts.append(inst)
            bb["instructions"] = new_insts
    return bir


def _patch_nc(nc):
    orig = nc.to_json_bytes

    def patched() -> bytes:
        return json.dumps(_split_excess_waits(json.loads(orig()))).encode()

    nc.to_json_bytes = patched
    return nc


# ---------------------------------------------------------------------------
# Device program (identical on all 8 cores; only DRAM contents differ)
# ---------------------------------------------------------------------------


def _build_nc():
    nc = bass.Bass()

    xt = nc.dram_tensor("xt", [D, NI], F32, kind="ExternalInput")
    yt = nc.dram_tensor("yt", [D, MJ], F32, kind="ExternalInput")
    lh = nc.dram_tensor("lh", [NCHUNK, P], F32, kind="ExternalInput")
    out = nc.dram_tensor("out", [NI, MJ], F16, kind="ExternalOutput")

    xt_r = xt.rearrange("(c d) i -> d c i", d=P)
    yt_r = yt.rearrange("(c d) j -> d c j", d=P)

    with tile.TileContext(nc) as tc:
        with (
            tc.tile_pool(name="singles", bufs=1) as singles,
            tc.tile_pool(name="stage", bufs=6) as stage,
            tc.tile_pool(name="sqp", bufs=2) as sqp,
            tc.tile_pool(name="outp", bufs=3) as outp,
            tc.tile_pool(name="tmpp", bufs=3) as tmpp,
        ):
            # persistent SBUF tensors
            xs8 = singles.tile([P, NCHUNK, NI], FP8)  # ih * x^T, fp8
            ys8 = singles.tile([P, NCHUNK, MJ], FP8)  # -2 ih * y^T, fp8
            mhx2a = singles.tile([P, ITILES // 2], F32)  # -0.5*x2 bias, slab0
            mhx2b = singles.tile([P, ITILES // 2], F32)  # ... slab 1
            sxa = singles.tile([1, SLABW], F32)  # -0.5 * x2 row, slab 0
            sxb = singles.tile([1, SLABW], F32)  # ... slab 1
            ey2row = singles.tile([1, MJ], BF16)  # exp(-0.5*y2) row
            ey2rep = singles.tile([P, MJ], F16)  # ... replicated
            ones1 = singles.tile([1, P], BF16)  # replicate-matmul lhsT
            lhs = singles.tile([P, NCHUNK], F32)
            ih = singles.tile([P, NCHUNK], F32)
            ihm2 = singles.tile([P, NCHUNK], F32)
            ihsq = singles.tile([P, NCHUNK], BF16)  # ih^2 reduce lhsT
            mihsq = singles.tile([P, NCHUNK], BF16)  # -0.5 ih^2 reduce lhsT

            nc.sync.dma_start(out=lhs, in_=lh.rearrange("c d -> d c"))
            nc.scalar.activation(ih, lhs, AF.Exp, scale=-0.5)
            nc.vector.tensor_scalar_mul(ihm2, ih, -2.0)
            nc.vector.tensor_mul(ihsq, ih, ih)
            nc.vector.tensor_scalar_mul(mihsq, ihsq, -0.5)
            nc.vector.memset(ones1, 1.0)

            held = {}  # x-slab-1 f32 chunk tiles, converted in main phase

            # ---- prep: loads, bf16 squares (DVE), bf16 row-reduce matmuls,
            # fp8 conversions (ACT; x-slab 1 deferred to the main phase) ----
            with tc.tile_pool(name="accp", bufs=1, space="PSUM") as accp:

                def prep_slab(src_r, s0, dst8, scale, lhsT, pfx, defer):
                    accs = [
                        accp.tile([1, 512], F32, tag=f"a{js}", name=f"{pfx}a{js}")
                        for js in range(SLABW // 512)
                    ]
                    for c in range(NCHUNK):
                        sf = stage.tile(
                            [P, SLABW], F32, tag="sf", name=f"{pfx}f{c}"
                        )
                        nc.sync.dma_start(
                            out=sf, in_=src_r[:, c, s0 : s0 + SLABW]
                        )
                        sq = sqp.tile(
                            [P, SLABW], BF16, tag="sq", name=f"{pfx}sq{c}"
                        )
                        nc.vector.tensor_mul(sq, sf, sf)
                        for js in range(SLABW // 512):
                            nc.tensor.matmul(
                                accs[js],
                                lhsT[:, c : c + 1],
                                sq[:, js * 512 : (js + 1) * 512],
                                start=(c == 0),
                                stop=(c == NCHUNK - 1),
                            )
                        if defer:
                            held[c] = sf
                        else:
                            nc.scalar.mul(
                                dst8[:, c, s0 : s0 + SLABW],
                                sf,
                                scale[:, c : c + 1],
                            )
                    return accs

                # y: ys8 + exp(-0.5*y2) row
                accs = prep_slab(yt_r, 0, ys8, ihm2, ihsq, "y", False)
                for js, acc in enumerate(accs):
                    nc.scalar.activation(
                        ey2row[0:1, js * 512 : (js + 1) * 512],
                        acc,
                        AF.Exp,
                        scale=-0.5,
                    )

                # x slab 0: xs8 + -0.5*x2 row
                accs = prep_slab(xt_r, 0, xs8, ih, mihsq, "x0", False)
                for js, acc in enumerate(accs):
                    nc.scalar.copy(sxa[0:1, js * 512 : (js + 1) * 512], acc)

                # x slab 1: loads only — squares/reduce/conversion all run
                # in the main phase so accp can close (freeing its PSUM
                # banks) as soon as y + x-slab-0 are reduced, letting the
                # main loop start ~14us earlier.
                for c in range(NCHUNK):
                    sf = stage.tile([P, SLABW], F32, tag="sf", name=f"x1f{c}")
                    nc.sync.dma_start(
                        out=sf, in_=xt_r[:, c, SLABW : 2 * SLABW]
                    )
                    held[c] = sf

            # transpose -0.5*x2 row (slab 0) -> [P, 16] for the ACT bias:
            # one column DMA per itile (anything wider needs >3 AP dims,
            # which the DMA engine can't express).
            for it in range(ITILES // 2):
                nc.sync.dma_start(
                    out=mhx2a[:, it : it + 1],
                    in_=sxa[0:1, it * P : (it + 1) * P],
                )

            sq1 = [
                singles.tile([P, SLABW], BF16, name=f"sq1_{c}")
                for c in range(NCHUNK)
            ]

            with tc.tile_pool(name="mainps", bufs=2, space="PSUM") as mainps:
                # replicate ey2row to all 128 partitions with ones-matmuls
                rep = mainps.tile([P, MJ], F32, tag="ps", name="rep")
                for h in range(MJ // 512):
                    nc.tensor.matmul(
                        rep[:, h * 512 : (h + 1) * 512],
                        ones1,
                        ey2row[0:1, h * 512 : (h + 1) * 512],
                        start=True,
                        stop=True,
                    )
                nc.vector.tensor_copy(ey2rep, rep)

                # ---- main loop: fp8 DR matmuls, ACT exp, DVE scale; the
                # x-slab-1 prep (fp8 conversion, squares, x2 reduction into
                # a borrowed PSUM ring slot) is software-pipelined into the
                # first itiles, whose outputs only need slab 0 ----
                for it in range(ITILES):
                    if 2 <= it < 2 + NCHUNK:
                        c = it - 2
                        nc.vector.tensor_scalar_mul(
                            xs8[:, c, SLABW : 2 * SLABW],
                            held[c],
                            ih[:, c : c + 1],
                        )
                        nc.vector.tensor_mul(sq1[c], held[c], held[c])
                    if it == 2 + NCHUNK:
                        # -0.5*x2 for slab 1, reduced in a borrowed slot
                        xacc = mainps.tile([P, MJ], F32, tag="ps", name="xacc")
                        for js in range(SLABW // 512):
                            jsl = slice(js * 512, (js + 1) * 512)
                            for c in range(NCHUNK):
                                nc.tensor.matmul(
                                    xacc[0:1, jsl],
                                    mihsq[:, c : c + 1],
                                    sq1[c][:, jsl],
                                    start=(c == 0),
                                    stop=(c == NCHUNK - 1),
                                )
                            nc.scalar.copy(sxb[0:1, jsl], xacc[0:1, jsl])
                        for itc2 in range(ITILES // 2):
                            nc.sync.dma_start(
                                out=mhx2b[:, itc2 : itc2 + 1],
                                in_=sxb[0:1, itc2 * P : (itc2 + 1) * P],
                            )
                    isl = slice(it * P, (it + 1) * P)
                    mhx2 = mhx2a if it < ITILES // 2 else mhx2b
                    itc = it % (ITILES // 2)
                    ps = mainps.tile([P, MJ], F32, tag="ps", name=f"ps{it}")
                    for t in range(NPAIR):
                        csl = slice(2 * t, 2 * t + 2)
                        for js in range(MJ // 512):
                            j0 = js * 512
                            nc.tensor.matmul(
                                ps[:, j0 : j0 + 512],
                                xs8[:, csl, isl],
                                ys8[:, csl, j0 : j0 + 512],
                                start=(t == 0),
                                stop=(t == NPAIR - 1),
                                perf_mode=DR,
                            )
                    tmp = tmpp.tile([P, MJ], F16, tag="tmp", name=f"t{it}")
                    nc.scalar.activation(
                        tmp,
                        ps,
                        AF.Exp,
                        bias=mhx2[:, itc : itc + 1],
                        scale=-0.5,
                    )
                    ot = outp.tile([P, MJ], F16, tag="ot", name=f"ot{it}")
                    nc.vector.tensor_mul(ot, tmp, ey2rep)
                    nc.sync.dma_start(out=out[isl, :], in_=ot)

    return _patch_nc(nc)


_NC_CACHE = None

# test.py hooks: set _TRACE to capture a profile; results object stored here.
_TRACE = False
_TRACE_KWARGS = {}
LAST_RESULTS = None


def kernel(x, y, logh):
    global _NC_CACHE, LAST_RESULTS
    x = np.ascontiguousarray(np.asarray(x, dtype=np.float32))
    y = np.ascontiguousarray(np.asarray(y, dtype=np.float32))
    logh = np.ascontiguousarray(np.asarray(logh, dtype=np.float32))
    assert x.shape == (N, D) and y.shape == (M, D) and logh.shape == (D,)

    if _NC_CACHE is None:
        _NC_CACHE = _build_nc()
    nc = _NC_CACHE

    lhm = np.ascontiguousarray(logh.reshape(NCHUNK, P))
    xts = [
        np.ascontiguousarray(x[r * NI : (r + 1) * NI, :].T) for r in range(RG)
    ]
    yts = [
        np.ascontiguousarray(y[q * MJ : (q + 1) * MJ, :].T) for q in range(CG)
    ]
    in_maps = []
    for c in range(N_CORES):
        r, q = divmod(c, CG)
        in_maps.append({"xt": xts[r], "yt": yts[q], "lh": lhm})

    res = run_bass_kernel_spmd(
        nc,
        in_maps,
        core_ids=list(range(N_CORES)),
        trace=_TRACE,
        **_TRACE_KWARGS,
    )
    LAST_RESULTS = res
    full = np.empty((N, M), dtype=np.float32)
    for c in range(N_CORES):
        r, q = divmod(c, CG)
        full[r * NI : (r + 1) * NI, q * MJ : (q + 1) * MJ] = res.results[c][
            "out"
        ].astype(np.float32)
    return full
